# revision 1
# baseline (speedup 1.0000x reference)
"""GCN 2-layer kernel for Trainium2, 8 NeuronCores.

Architecture: 3 SPMD dispatches, all gathers done on host (index movement).
  - Shard by destination-node range: core c owns dst nodes [c*12544, (c+1)*12544).
  - Edges (incl. self-loops) are routed to the dst-owning core, sorted by dst,
    packed into 128-edge chunks targeting 32-node destination windows
    (global static schedule so all cores run identical code).
  - Degree is a host-side bincount of dst indices (part of schedule build);
    all FLOAT math (rsqrt etc.) stays on device.
  - d0 (tiny): dis = sqrt(1/deg), sq = sqrt(deg), ideg = 1/deg, xs = x*dis
    (bf16). Host gathers xs[src] per edge.
  - d2: one-hot cmp = (iota == dst_rel) in bf16 (DVE); scatter-add via
    col-tiled bf16 matmuls (4 concurrent 32-partition groups); psum holds
    raw aggregate A_raw[f, n] = sum_e xs_src.  The dis_d scaling is folded
    through relu via  relu(dis*x + b1) = dis*relu(x + sqrt(deg)*b1):
    phase B computes h1t = relu([W1; b1]^T @ [A_raw; sq]) (K=17 matmul),
    phase C computes zs = ideg * (h1t^T @ W2) with a per-partition scalar.
    cmp tiles are streamed out to HBM for d3 to reuse.
  - host: gathers zs[src] per edge (bf16).
  - d3: scatter-add zs_src via the HBM-cached cmp one-hots (no compares);
    out = dis_d * agg + b2 via two [128,196] tensor ops.
"""
import sys

sys.path.insert(0, '/opt/trn_rl_repo')

import numpy as np
import concourse.bass as bass
import concourse.tile as tile
from concourse import bacc, mybir
from concourse.bass_utils import run_bass_kernel_spmd

N_NODES = 100000
N_CORES = 8
NPC = 12544            # nodes per core = 98 * 128
NPAD = NPC * N_CORES   # 100352
W = 32                 # dst window width
NWIN = NPC // W        # 392 windows per core
NGRP = 4               # psum col groups (tile_position col tiling)
WPB = 64               # windows per psum bank (16 per group x 4 groups)
NBANKA = (NWIN + WPB - 1) // WPB   # 7 aggregation banks
NCOLS = NPC // 128     # 98 (wrap columns / 128-node slices)
NSB = (NCOLS + 3) // 4  # 25 superblocks of 512 nodes
F_IN = 16
F_HID = 128
F_OUT = 2
CHUNK = 128
CMP_BATCH = 32         # slots per compare op
DT = mybir.dt.float32
BF = mybir.dt.float16
NP_BF = np.float16


# ---------------------------------------------------------------- host prep

def build_schedule(edge_index):
    """Partition + sort edges, build the global static slot schedule and the
    per-node degree histogram (host-side integer index counting)."""
    src = np.asarray(edge_index[0])
    dst = np.asarray(edge_index[1])

    # degree histogram over dst (+1 self-loop per real node); pads get deg=1
    deg = np.bincount(dst, minlength=NPAD).astype(np.int64)
    deg[:N_NODES] += 1
    deg[N_NODES:] = 1

    per_core = []
    counts = np.zeros((N_CORES, NWIN), dtype=np.int64)
    for c in range(N_CORES):
        lo, hi = c * NPC, (c + 1) * NPC
        sel = (dst >= lo) & (dst < hi)
        es = src[sel].astype(np.int64)
        ed = (dst[sel] - lo).astype(np.int64)
        n_real = min(hi, N_NODES) - lo
        self_d = np.arange(n_real, dtype=np.int64)
        es = np.concatenate([es, self_d + lo])
        ed = np.concatenate([ed, self_d])
        order = np.argsort(ed, kind='stable')
        es, ed = es[order], ed[order]
        win = ed // W
        counts[c] = np.bincount(win, minlength=NWIN)
        per_core.append((es, ed))

    k_w = np.ceil(counts.max(axis=0) / CHUNK).astype(np.int64)
    k_w = np.maximum(k_w, 1)
    S_real = int(k_w.sum())
    S = ((S_real + CMP_BATCH - 1) // CMP_BATCH) * CMP_BATCH  # pad to batch mult

    # schedule: per slot -> (window, is_first_chunk_of_window, is_last)
    sched = []
    for w in range(NWIN):
        for k in range(int(k_w[w])):
            sched.append((w, k == 0, k == int(k_w[w]) - 1))

    # per-(bank, group) first/last slot, per-bank last slot
    first_bg, last_bg, last_bank = {}, {}, {}
    for s, (w, fc, lc) in enumerate(sched):
        bank, grp = w // WPB, w % NGRP
        if (bank, grp) not in first_bg:
            first_bg[(bank, grp)] = s
        last_bg[(bank, grp)] = s
        last_bank[bank] = s

    # per-core arrays [128, S]
    srcidx = np.zeros((N_CORES, S, CHUNK), dtype=np.int64)
    valid = np.zeros((N_CORES, S, CHUNK), dtype=bool)
    dst_rel = np.full((N_CORES, S, CHUNK), -1.0, dtype=np.float32)
    for c in range(N_CORES):
        es, ed = per_core[c]
        starts = np.zeros(NWIN + 1, dtype=np.int64)
        np.cumsum(counts[c], out=starts[1:])
        slot = 0
        for w in range(NWIN):
            e0, e1 = int(starts[w]), int(starts[w + 1])
            for k in range(int(k_w[w])):
                a = e0 + k * CHUNK
                b = min(e0 + (k + 1) * CHUNK, e1)
                m = max(0, b - a)
                if m > 0:
                    srcidx[c, slot, :m] = es[a:b]
                    valid[c, slot, :m] = True
                    dst_rel[c, slot, :m] = (ed[a:b] - w * W).astype(np.float32)
                slot += 1
        assert slot == S_real

    srcidx_t = np.ascontiguousarray(srcidx.transpose(0, 2, 1))      # [C,128,S]
    valid_t = np.ascontiguousarray(valid.transpose(0, 2, 1))
    dst_rel_t = np.ascontiguousarray(dst_rel.transpose(0, 2, 1))

    iota = np.tile(np.arange(W, dtype=np.float32), CMP_BATCH)       # [1024]
    iota_tiled = np.ascontiguousarray(np.broadcast_to(iota, (CHUNK, W * CMP_BATCH)))

    return dict(S=S, S_real=S_real, sched=sched, srcidx=srcidx_t, valid=valid_t,
                dst_rel=dst_rel_t, iota_tiled=iota_tiled, deg=deg,
                first_bg=first_bg, last_bg=last_bg, last_bank=last_bank)


def gather_rows(table, srcidx, valid, f):
    """host gather: msg[c, p, s*f:(s+1)*f] = table[srcidx[c,p,s]] (0 if pad)."""
    C, P, S = srcidx.shape
    out = table[srcidx.reshape(-1)].reshape(C, P, S, f)
    out[~valid] = 0
    return np.ascontiguousarray(out.reshape(C, P, S * f))


def wrap2(v):
    """[NPC] -> [128, 98] wrap layout (n = c*128 + p)."""
    return np.ascontiguousarray(v.reshape(NCOLS, 128).T)


def unwrap2(m):
    """[128, 98] -> [NPC]"""
    return np.ascontiguousarray(m.T.reshape(-1))


# ------------------------------------------------------------- bass helpers

def new_nc():
    return bacc.Bacc('TRN2', target_bir_lowering=False, debug=False,
                     num_devices=N_CORES)


# --------------------------------------------------------------- program d0

def build_d0():
    """dis = sqrt(1/deg); sq = sqrt(deg); ideg = 1/deg; xs = x * dis (bf16)."""
    nc = new_nc()
    x_in = nc.dram_tensor('x_wrap', [CHUNK, NCOLS * F_IN], DT, kind='ExternalInput')
    deg_in = nc.dram_tensor('deg_wrap', [CHUNK, NCOLS], DT, kind='ExternalInput')
    xs_out = nc.dram_tensor('xs_bf', [CHUNK, NCOLS * F_IN], BF, kind='ExternalOutput')
    dis_out = nc.dram_tensor('dis', [CHUNK, NCOLS], DT, kind='ExternalOutput')
    sq_out = nc.dram_tensor('sq_bf', [CHUNK, NCOLS], BF, kind='ExternalOutput')
    ideg_out = nc.dram_tensor('ideg', [CHUNK, NCOLS], DT, kind='ExternalOutput')

    with tile.TileContext(nc) as tc:
        with tc.tile_pool(name='p', bufs=1) as pp:
            x_t = pp.tile([CHUNK, NCOLS * F_IN], DT)
            nc.sync.dma_start(x_t[:], x_in.ap())
            deg_t = pp.tile([CHUNK, NCOLS], DT)
            nc.scalar.dma_start(deg_t[:], deg_in.ap())

            ideg_t = pp.tile([CHUNK, NCOLS], DT)
            nc.vector.reciprocal(ideg_t[:], deg_t[:])
            dis_t = pp.tile([CHUNK, NCOLS], DT)
            nc.scalar.sqrt(dis_t[:], ideg_t[:])
            sq_t = pp.tile([CHUNK, NCOLS], BF)
            nc.scalar.sqrt(sq_t[:], deg_t[:])

            xs_t = pp.tile([CHUNK, NCOLS * F_IN], BF)
            nc.vector.tensor_tensor(
                out=xs_t[:], in0=x_t[:],
                in1=dis_t[:].to_broadcast([CHUNK, NCOLS, F_IN]),
                op=mybir.AluOpType.mult)

            nc.sync.dma_start(xs_out.ap(), xs_t[:])
            nc.scalar.dma_start(dis_out.ap(), dis_t[:])
            nc.gpsimd.dma_start(sq_out.ap(), sq_t[:])
            nc.gpsimd.dma_start(ideg_out.ap(), ideg_t[:])

    nc.compile()
    return nc


# --------------------------------------------------------------- program d2

def build_d2(S, sched, first_bg, last_bg, last_bank):
    """Layer 1 + z:  A_raw scatter-add -> h1t = relu(W1b^T @ [A_raw; sq])
    -> zs = ideg * (h1t^T @ W2).  Streams cmp one-hots to HBM for d3."""
    nc = new_nc()
    dst_rel_in = nc.dram_tensor('dst_rel', [CHUNK, S], DT, kind='ExternalInput')
    iota_in = nc.dram_tensor('iota_tiled', [CHUNK, CMP_BATCH * W], DT,
                             kind='ExternalInput')
    xsrc_in = nc.dram_tensor('xs_src', [CHUNK, S * F_IN], BF, kind='ExternalInput')
    sq_in = nc.dram_tensor('sq_row', [1, NPC], BF, kind='ExternalInput')
    ideg_in = nc.dram_tensor('ideg', [CHUNK, NCOLS], DT, kind='ExternalInput')
    w1b_in = nc.dram_tensor('W1b', [F_IN + 1, F_HID], DT, kind='ExternalInput')
    w2_in = nc.dram_tensor('W2', [F_HID, F_OUT], DT, kind='ExternalInput')
    zs_out = nc.dram_tensor('zs_wrap', [CHUNK, F_OUT * NCOLS], BF,
                            kind='ExternalOutput')
    cmp_out = nc.dram_tensor('cmp_hbm', [CHUNK, S * W], BF, kind='ExternalOutput')

    n_batches = S // CMP_BATCH

    with tile.TileContext(nc) as tc:
        with tc.tile_pool(name='persist', bufs=1) as pp, \
             tc.tile_pool(name='cmp', bufs=6) as cmpp, \
             tc.tile_pool(name='msg', bufs=8) as msgp, \
             tc.tile_pool(name='aggps', bufs=3, space='PSUM') as aggps, \
             tc.tile_pool(name='h1ps', bufs=2, space='PSUM') as h1ps, \
             tc.tile_pool(name='zps', bufs=2, space='PSUM') as zps:
            dst_rel_t = pp.tile([CHUNK, S], DT)
            nc.scalar.dma_start(dst_rel_t[:], dst_rel_in.ap())
            iota_t = pp.tile([CHUNK, CMP_BATCH * W], DT)
            nc.sync.dma_start(iota_t[:], iota_in.ap())
            ideg_t = pp.tile([CHUNK, NCOLS], DT)
            nc.sync.dma_start(ideg_t[:], ideg_in.ap())
            w1b_f32 = pp.tile([F_IN + 1, F_HID], DT)
            nc.sync.dma_start(w1b_f32[:], w1b_in.ap())
            w1b_t = pp.tile([F_IN + 1, F_HID], BF)
            nc.vector.tensor_copy(w1b_t[:], w1b_f32[:])
            w2_f32 = pp.tile([F_HID, F_OUT], DT)
            nc.sync.dma_start(w2_f32[:], w2_in.ap())
            w2_t = pp.tile([F_HID, F_OUT], BF)
            nc.vector.tensor_copy(w2_t[:], w2_f32[:])

            # agg17[0:16] = raw aggregate (flushed from psum), agg17[16] = sq
            agg17 = pp.tile([F_IN + 1, NPC], BF)
            nc.sync.dma_start(agg17[F_IN:F_IN + 1, :], sq_in.ap())
            h1_sb = pp.tile([F_HID, NPC], BF)
            zs_sb = pp.tile([CHUNK, F_OUT * NCOLS], BF)

            agg_tiles = {}

            def flush_bank(bank):
                """psum bank -> agg17 rows 0..16 (ACT engine)."""
                w0 = bank * WPB
                nw = min(NWIN, w0 + WPB) - w0           # windows in bank
                nwg = nw // NGRP                        # per group
                at = agg_tiles[bank]
                for g in range(NGRP):
                    # agg17 cols for window w=4a+g, a in [16*bank, 16*bank+nwg)
                    view = (agg17[0:F_IN, 2048 * bank:2048 * bank + 128 * nwg]
                            .rearrange('p (a r) -> p a r', r=128)
                            [:, :, 32 * g:32 * g + W])
                    srcv = (at[32 * g:32 * g + F_IN, 0:32 * nwg]
                            .rearrange('p (a r) -> p a r', r=W))
                    nc.scalar.copy(view, srcv)

            def emit_B(bank):
                for k in range(4 * bank, min(4 * bank + 4, NSB)):
                    c0 = 512 * k
                    c1 = min(c0 + 512, NPC)
                    h1p = h1ps.tile([F_HID, 512], DT, space='PSUM', tag='h1')
                    nc.tensor.matmul(out=h1p[:, :c1 - c0], lhsT=w1b_t[:],
                                     rhs=agg17[:, c0:c1], start=True, stop=True)
                    nc.scalar.activation(h1_sb[:, c0:c1], h1p[:, :c1 - c0],
                                         mybir.ActivationFunctionType.Relu)

            def emit_C(bank):
                for k in range(4 * bank, min(4 * bank + 4, NSB)):
                    c0 = 512 * k
                    c1 = min(c0 + 512, NPC)
                    for sl in range(c0 // 128, c1 // 128):
                        zp = zps.tile([CHUNK, F_OUT], DT, space='PSUM', tag='z')
                        nc.tensor.matmul(out=zp[:],
                                         lhsT=h1_sb[:, sl * 128:(sl + 1) * 128],
                                         rhs=w2_t[:], start=True, stop=True)
                        nc.scalar.mul(zs_sb[:, sl * F_OUT:(sl + 1) * F_OUT],
                                      zp[:], ideg_t[:, sl:sl + 1])

            for b in range(n_batches):
                cmp_t = cmpp.tile([CHUNK, CMP_BATCH * W], BF, tag='cmp')
                nc.vector.tensor_tensor(
                    out=cmp_t[:],
                    in0=iota_t[:],
                    in1=dst_rel_t[:, b * CMP_BATCH:(b + 1) * CMP_BATCH]
                        .to_broadcast([CHUNK, CMP_BATCH, W]),
                    op=mybir.AluOpType.is_equal)
                nc.gpsimd.dma_start(
                    cmp_out.ap()[:, b * CMP_BATCH * W:(b + 1) * CMP_BATCH * W],
                    cmp_t[:])
                msg_t = msgp.tile([CHUNK, CMP_BATCH * F_IN], BF, tag='msg')
                nc.sync.dma_start(
                    msg_t[:],
                    xsrc_in.ap()[:, b * CMP_BATCH * F_IN:(b + 1) * CMP_BATCH * F_IN])
                for j in range(CMP_BATCH):
                    s = b * CMP_BATCH + j
                    if s >= len(sched):
                        break
                    w, fc, lc = sched[s]
                    bank, grp = w // WPB, w % NGRP
                    colb = 32 * ((w // NGRP) % (WPB // NGRP))
                    if bank not in agg_tiles:
                        agg_tiles[bank] = aggps.tile(
                            [CHUNK, 512], DT, space='PSUM', tag='agg',
                            name=f'aggbank{bank}')
                    nc.tensor.matmul(
                        out=agg_tiles[bank][32 * grp:32 * grp + F_IN,
                                            colb:colb + W],
                        lhsT=msg_t[:, j * F_IN:(j + 1) * F_IN],
                        rhs=cmp_t[:, j * W:(j + 1) * W],
                        start=(s == first_bg[(bank, grp)]),
                        stop=(s == last_bg[(bank, grp)]),
                        tile_position=(0, 32 * grp),
                    )
                    if s == last_bank[bank]:
                        flush_bank(bank)
                        if bank >= 1:
                            emit_B(bank - 1)
                        if bank >= 2:
                            emit_C(bank - 2)

            emit_B(NBANKA - 1)
            emit_C(NBANKA - 2)
            emit_C(NBANKA - 1)
            nc.sync.dma_start(zs_out.ap(), zs_sb[:])

    nc.compile()
    return nc


# --------------------------------------------------------------- program d3

def build_d3(S, sched):
    """Layer 2 aggregation from HBM-cached one-hots:
    out = dis_d * scatter(zs_src) + b2."""
    nc = new_nc()
    cmp_in = nc.dram_tensor('cmp_hbm', [CHUNK, S * W], BF, kind='ExternalInput')
    zssrc_in = nc.dram_tensor('zs_src', [CHUNK, S * F_OUT], BF,
                              kind='ExternalInput')
    drep_in = nc.dram_tensor('drep', [CHUNK, F_OUT * NWIN // NGRP], DT,
                             kind='ExternalInput')
    b2rep_in = nc.dram_tensor('b2rep', [CHUNK, F_OUT * NWIN // NGRP], DT,
                              kind='ExternalInput')
    out_out = nc.dram_tensor('out_wrap4', [CHUNK, F_OUT * NWIN // NGRP], DT,
                             kind='ExternalOutput')

    NCOL3 = F_OUT * NWIN // NGRP    # 196
    n_batches = S // CMP_BATCH

    with tile.TileContext(nc) as tc:
        with tc.tile_pool(name='persist', bufs=1) as pp, \
             tc.tile_pool(name='cmp', bufs=8) as cmpp, \
             tc.tile_pool(name='msg', bufs=8) as msgp, \
             tc.tile_pool(name='psum', bufs=1, space='PSUM') as psp, \
             tc.tile_pool(name='outp', bufs=1) as outp:
            drep_t = pp.tile([CHUNK, NCOL3], DT)
            nc.sync.dma_start(drep_t[:], drep_in.ap())
            b2rep_t = pp.tile([CHUNK, NCOL3], DT)
            nc.sync.dma_start(b2rep_t[:], b2rep_in.ap())

            out_ps = psp.tile([CHUNK, NCOL3], DT, space='PSUM')

            for b in range(n_batches):
                cmp_t = cmpp.tile([CHUNK, CMP_BATCH * W], BF, tag='cmp')
                eng = nc.sync if b % 2 == 0 else nc.scalar
                eng.dma_start(
                    cmp_t[:],
                    cmp_in.ap()[:, b * CMP_BATCH * W:(b + 1) * CMP_BATCH * W])
                msg_t = msgp.tile([CHUNK, CMP_BATCH * F_OUT], BF, tag='msg')
                nc.sync.dma_start(
                    msg_t[:],
                    zssrc_in.ap()[:, b * CMP_BATCH * F_OUT:(b + 1) * CMP_BATCH * F_OUT])
                for j in range(CMP_BATCH):
                    s = b * CMP_BATCH + j
                    if s >= len(sched):
                        break
                    w, fc, lc = sched[s]
                    grp = w % NGRP
                    col = F_OUT * (w // NGRP)
                    nc.tensor.matmul(
                        out=out_ps[32 * grp:32 * grp + W, col:col + F_OUT],
                        lhsT=cmp_t[:, j * W:(j + 1) * W],
                        rhs=msg_t[:, j * F_OUT:(j + 1) * F_OUT],
                        start=(s == _d3_first[grp]),
                        stop=(s == _d3_last[grp]),
                        tile_position=(0, 32 * grp),
                    )

            scaled = outp.tile([CHUNK, NCOL3], DT)
            nc.vector.tensor_tensor(out=scaled[:], in0=out_ps[:], in1=drep_t[:],
                                    op=mybir.AluOpType.mult)
            final = outp.tile([CHUNK, NCOL3], DT)
            nc.vector.tensor_tensor(out=final[:], in0=scaled[:], in1=b2rep_t[:],
                                    op=mybir.AluOpType.add)
            nc.sync.dma_start(out_out.ap(), final[:])

    nc.compile()
    return nc


_d3_first = {}
_d3_last = {}


def prep_d3_groups(sched):
    _d3_first.clear()
    _d3_last.clear()
    for s, (w, fc, lc) in enumerate(sched):
        g = w % NGRP
        if g not in _d3_first:
            _d3_first[g] = s
        _d3_last[g] = s


# ------------------------------------------------------------------ runner

RESULTS = []  # BassKernelResults of the last run (for profiling)


def run_gcn(x, edge_index, W1, b1, W2, b2, trace=False):
    x = np.asarray(x, dtype=np.float32)
    W1 = np.asarray(W1, dtype=np.float32)
    b1 = np.asarray(b1, dtype=np.float32)
    W2 = np.asarray(W2, dtype=np.float32)
    b2 = np.asarray(b2, dtype=np.float32)

    sch = build_schedule(edge_index)
    S, sched = sch['S'], sch['sched']
    prep_d3_groups(sched)
    print(f'[host] slots S={S} (real {sch["S_real"]}), '
          f'edges+selfloops={int(sch["valid"].sum())}')

    import time
    t0 = time.time()
    nc0 = build_d0()
    nc2 = build_d2(S, sched, sch['first_bg'], sch['last_bg'], sch['last_bank'])
    nc3 = build_d3(S, sched)
    print(f'[host] compiled in {time.time()-t0:.1f}s')

    core_ids = list(range(N_CORES))
    times = {}
    RESULTS.clear()

    # ---------- d0
    x_pad = np.zeros((NPAD, F_IN), dtype=np.float32)
    x_pad[:N_NODES] = x
    deg_f = sch['deg'].astype(np.float32)
    in0 = []
    for c in range(N_CORES):
        lo = c * NPC
        xw = np.ascontiguousarray(
            x_pad[lo:lo + NPC].reshape(NCOLS, 128, F_IN).transpose(1, 0, 2)
            .reshape(CHUNK, NCOLS * F_IN))
        in0.append({'x_wrap': xw, 'deg_wrap': wrap2(deg_f[lo:lo + NPC])})
    r0 = run_bass_kernel_spmd(nc0, in0, core_ids=core_ids, trace=trace)
    RESULTS.append(r0)
    times['d0'] = r0.exec_time_ns

    xs_full = np.zeros((NPAD, F_IN), dtype=NP_BF)
    dis_full = np.empty(NPAD, dtype=np.float32)
    sq_full = np.empty(NPAD, dtype=NP_BF)
    for c in range(N_CORES):
        lo = c * NPC
        xs_full[lo:lo + NPC] = (r0.results[c]['xs_bf']
                                .reshape(CHUNK, NCOLS, F_IN).transpose(1, 0, 2)
                                .reshape(NPC, F_IN))
        dis_full[lo:lo + NPC] = unwrap2(r0.results[c]['dis'])
        sq_full[lo:lo + NPC] = unwrap2(r0.results[c]['sq_bf'])
    xs_full[N_NODES:] = 0

    # ---------- host gather (index movement only)
    xs_src = gather_rows(xs_full, sch['srcidx'], sch['valid'], F_IN)

    W1b = np.concatenate([W1, b1[None, :]], axis=0)  # [17, 128]

    # ---------- d2
    in2 = []
    for c in range(N_CORES):
        lo = c * NPC
        in2.append({
            'dst_rel': sch['dst_rel'][c], 'iota_tiled': sch['iota_tiled'],
            'xs_src': xs_src[c],
            'sq_row': np.ascontiguousarray(sq_full[lo:lo + NPC])[None, :],
            'ideg': r0.results[c]['ideg'],
            'W1b': W1b, 'W2': W2,
        })
    r2 = run_bass_kernel_spmd(nc2, in2, core_ids=core_ids, trace=trace)
    RESULTS.append(r2)
    times['d2'] = r2.exec_time_ns

    zs_full = np.zeros((NPAD, F_OUT), dtype=NP_BF)
    for c in range(N_CORES):
        lo = c * NPC
        zs_full[lo:lo + NPC] = (r2.results[c]['zs_wrap']
                                .reshape(CHUNK, NCOLS, F_OUT).transpose(1, 0, 2)
                                .reshape(NPC, F_OUT))
    zs_full[N_NODES:] = 0

    zs_src = gather_rows(zs_full, sch['srcidx'], sch['valid'], F_OUT)

    # ---------- d3
    # drep[32j+r, 2a+f] = dis[32(4a+j)+r];  b2rep[p, 2a+f] = b2[f]
    NCOL3 = F_OUT * NWIN // NGRP
    jj, rr = np.divmod(np.arange(CHUNK), W)       # p = 32j+r
    aa = np.arange(NWIN // NGRP)
    loc = (32 * (4 * aa[None, :] + jj[:, None]) + rr[:, None])  # [128, 98]
    b2rep = np.ascontiguousarray(
        np.broadcast_to(b2[None, None, :], (CHUNK, NWIN // NGRP, F_OUT))
        .reshape(CHUNK, NCOL3)).astype(np.float32)
    in3 = []
    for c in range(N_CORES):
        lo = c * NPC
        drep = np.repeat(dis_full[lo:lo + NPC][loc], F_OUT, axis=1) \
            .reshape(CHUNK, NCOL3).astype(np.float32)
        in3.append({
            'cmp_hbm': r2.results[c]['cmp_hbm'],
            'zs_src': zs_src[c],
            'drep': np.ascontiguousarray(drep),
            'b2rep': b2rep,
        })
    r3 = run_bass_kernel_spmd(nc3, in3, core_ids=core_ids, trace=trace)
    RESULTS.append(r3)
    times['d3'] = r3.exec_time_ns

    out_full = np.empty((NPAD, F_OUT), dtype=np.float32)
    for c in range(N_CORES):
        ow = r3.results[c]['out_wrap4']            # [128, 196]
        # local n = 32w+r, w = 4a+j -> p = 32j+r, col = 2a+f
        n = np.arange(NPC)
        wv, rv = np.divmod(n, W)
        jv, av = wv % NGRP, wv // NGRP
        out_full[c * NPC:(c + 1) * NPC, 0] = ow[32 * jv + rv, 2 * av]
        out_full[c * NPC:(c + 1) * NPC, 1] = ow[32 * jv + rv, 2 * av + 1]
    return out_full[:N_NODES].astype(np.float32), times


# ------------------------------------------------------------- entry point

TRACE = False
LAST_TIMES = {}


def kernel(x, edge_index, W1, b1, W2, b2):
    """Full-input GCN kernel: shards across 8 NeuronCores internally."""
    global LAST_TIMES
    out, times = run_gcn(x, edge_index, W1, b1, W2, b2, trace=TRACE)
    LAST_TIMES = times
    return out



# revision 3
# speedup vs baseline: 2.5839x; 2.5839x over previous
"""GCN 2-layer kernel for Trainium2, 8 NeuronCores.

Architecture: 3 SPMD dispatches; all gathers/index work on host.
  - Shard by destination-node range: core c owns dst nodes [c*12544, (c+1)*12544).
  - d0: dis = sqrt(1/deg) (deg from host bincount), xs = x*dis in fp16.
  - Host gathers xs[src] per edge into a degree-padded layout: each core's
    nodes are sorted by degree (desc); rank r -> (group g=r%8, pos=r//8);
    partition 16g+f holds feature f of group g; the free axis is split into
    degree classes (pos ranges sharing a padded width D).  Padding ~2.3%.
  - d2: per-class strided tensor_reduce (DVE) sums each node's messages ->
    A1; scale by dis_dst (Pool); SBUF->SBUF DMA reshuffles A to [16, 12544];
    h1 = relu(W1^T A + b1) via K=16 matmuls + ACT eviction; z = per-128-col
    swapped matmuls (lhsT=h1 block, rhs=W2) giving [128, 196] psum, scaled
    by dis_node -> zs fp16.
  - Host gathers zs[src] per edge into the d3 padded layout (rank r ->
    (p=r%128, pos=r//128), features interleaved mid-axis).
  - d3: per-class tensor_reduce -> A2; out = dis_dst*A2 + b2.
"""
import sys

sys.path.insert(0, '/opt/trn_rl_repo')

import numpy as np
import concourse.bass as bass
import concourse.tile as tile
from concourse import bacc, mybir
from concourse.bass_utils import run_bass_kernel_spmd

N_NODES = 100000
N_CORES = 8
NPC = 12544             # nodes per core = 98 * 128
NPAD = NPC * N_CORES    # 100352
NPOS2 = NPC // 8        # 1568 positions per group (d2)
NPOS3 = NPC // 128      # 98 positions (d3)
NCOLS = NPC // 128      # 98 wrap columns
F_IN = 16
F_HID = 128
F_OUT = 2
K2 = 12                 # degree classes for d2
K3 = 6                  # degree classes for d3
CH2 = 3584              # max free elems per d2 DMA/reduce unit
DT = mybir.dt.float32
BF = mybir.dt.float16
NP_BF = np.float16


# ---------------------------------------------------------------- host prep

def dp_classes(w, K):
    """Split desc-sorted widths w into <=K contiguous classes minimizing
    sum(n_k * D_k) with D_k = w[class start].  Returns [(P0, n, D)]."""
    w = np.maximum(np.asarray(w, dtype=np.int64), 1)
    P = len(w)
    INF = float('inf')
    dp = np.full((K + 1, P + 1), INF)
    dp[0, 0] = 0.0
    choice = np.zeros((K + 1, P + 1), dtype=np.int64)
    for k in range(1, K + 1):
        for p in range(1, P + 1):
            q = np.arange(p)
            costs = dp[k - 1, :p] + (p - q) * w[q]
            i = int(np.argmin(costs))
            dp[k, p] = costs[i]
            choice[k, p] = i
    cls = []
    p = P
    for k in range(K, 0, -1):
        q = int(choice[k, p])
        if p > q:
            cls.append((q, p - q, int(w[q])))
        p = q
    return cls[::-1]


def build_schedule(edge_index):
    """Edge partition + degree-sorted padded-layout schedule (host ints)."""
    src = np.asarray(edge_index[0]).astype(np.int64)
    dst = np.asarray(edge_index[1]).astype(np.int64)

    deg = np.bincount(dst, minlength=NPAD).astype(np.int64)
    deg[:N_NODES] += 1          # self-loops
    deg[N_NODES:] = 0           # pads: no edges (deg input to d0 is 1)

    cores = []
    for c in range(N_CORES):
        lo, hi = c * NPC, (c + 1) * NPC
        sel = (dst >= lo) & (dst < hi)
        es = src[sel]
        ed = dst[sel] - lo
        n_real = min(hi, N_NODES) - lo
        loop_d = np.arange(n_real, dtype=np.int64)
        es = np.concatenate([es, loop_d + lo])
        ed = np.concatenate([ed, loop_d])
        order = np.argsort(ed, kind='stable')
        es = es[order]                       # global src ids, dst-sorted
        cnt = np.bincount(ed, minlength=NPC)
        starts = np.zeros(NPC + 1, dtype=np.int64)
        np.cumsum(cnt, out=starts[1:])
        degs = deg[lo:hi]
        rank_nodes = np.argsort(-degs, kind='stable')   # local ids by rank
        cores.append(dict(es=es, starts=starts, cnt=cnt,
                          rank_nodes=rank_nodes, lo=lo))

    # shared class widths: max over cores at each rank position
    deg_sorted = np.stack([deg[c['lo']:c['lo'] + NPC][c['rank_nodes']]
                           for c in cores])             # [8, NPC] desc
    p2 = deg_sorted.reshape(N_CORES, NPOS2, 8).max(axis=2).max(axis=0)
    p3 = deg_sorted.reshape(N_CORES, NPOS3, 128).max(axis=2).max(axis=0)
    cls2 = dp_classes(p2, K2)
    cls3 = dp_classes(p3, K3)
    tot2 = sum(n * D for _, n, D in cls2)
    tot3 = sum(n * D for _, n, D in cls3) * F_OUT
    return dict(cores=cores, deg=deg, cls2=cls2, cls3=cls3,
                tot2=tot2, tot3=tot3)


def class_offsets(cls):
    offs, o = [], 0
    for _, n, D in cls:
        offs.append(o)
        o += n * D
    return offs


def build_idx2(sch, c):
    """Per-class gather indices for d2: list of [8, n, D] into table rows
    (NPAD = zero row)."""
    co = sch['cores'][c]
    rank_nodes, starts, cnt, es = (co['rank_nodes'], co['starts'],
                                   co['cnt'], co['es'])
    out = []
    for P0, n, D in sch['cls2']:
        r = (8 * (P0 + np.arange(n))[None, :, None]
             + np.arange(8)[:, None, None])            # [8, n, 1]
        nodes = rank_nodes[r[..., 0]]                  # [8, n]
        base = starts[nodes][..., None]                # [8, n, 1]
        j = np.arange(D)[None, None, :]
        valid = j < cnt[nodes][..., None]
        eidx = np.where(valid, base + j, 0)
        idx = np.where(valid, es[eidx], NPAD)
        out.append(idx)
    return out


def build_idx3(sch, c):
    """Per-class gather indices for d3: list of [128, n, D]."""
    co = sch['cores'][c]
    rank_nodes, starts, cnt, es = (co['rank_nodes'], co['starts'],
                                   co['cnt'], co['es'])
    out = []
    for P0, n, D in sch['cls3']:
        r = (128 * (P0 + np.arange(n))[None, :, None]
             + np.arange(128)[:, None, None])          # [128, n, 1]
        nodes = rank_nodes[r[..., 0]]
        base = starts[nodes][..., None]
        j = np.arange(D)[None, None, :]
        valid = j < cnt[nodes][..., None]
        eidx = np.where(valid, base + j, 0)
        idx = np.where(valid, es[eidx], NPAD)
        out.append(idx)
    return out


def gather2(xs_full, idx2, tot2):
    """xs_pad [128, tot2]: partition 16g+f, per-class [n*D] blocks."""
    out = np.empty((128, tot2), dtype=NP_BF)
    o = 0
    for idx in idx2:
        _, n, D = idx.shape
        vals = xs_full[idx]                            # [8, n, D, 16]
        out[:, o:o + n * D] = (vals.transpose(0, 3, 1, 2)
                               .reshape(128, n * D))
        o += n * D
    return np.ascontiguousarray(out)


def gather3(zs_full, idx3, tot3):
    """zs_pad [128, tot3]: per-class [(n*2)*D] blocks, feat mid-axis."""
    out = np.empty((128, tot3), dtype=NP_BF)
    o = 0
    for idx in idx3:
        _, n, D = idx.shape
        vals = zs_full[idx]                            # [128, n, D, 2]
        out[:, o:o + n * 2 * D] = (vals.transpose(0, 1, 3, 2)
                                   .reshape(128, n * 2 * D))
        o += n * 2 * D
    return np.ascontiguousarray(out)


def wrap2(v):
    """[NPC] -> [128, 98] wrap layout (n = c*128 + p)."""
    return np.ascontiguousarray(v.reshape(NCOLS, 128).T)


def unwrap2(m):
    return np.ascontiguousarray(m.T.reshape(-1))


# ------------------------------------------------------------- bass helpers

def new_nc():
    return bacc.Bacc('TRN2', target_bir_lowering=False, debug=False,
                     num_devices=N_CORES)


# --------------------------------------------------------------- program d0

def build_d0():
    """dis = sqrt(1/deg) fp32+fp16; xs = x * dis (fp16)."""
    nc = new_nc()
    x_in = nc.dram_tensor('x_wrap', [128, NCOLS * F_IN], DT,
                          kind='ExternalInput')
    deg_in = nc.dram_tensor('deg_wrap', [128, NCOLS], DT,
                            kind='ExternalInput')
    xs_out = nc.dram_tensor('xs_bf', [128, NCOLS * F_IN], BF,
                            kind='ExternalOutput')
    dis_out = nc.dram_tensor('dis', [128, NCOLS], DT, kind='ExternalOutput')
    dis16_out = nc.dram_tensor('dis16', [128, NCOLS], BF,
                               kind='ExternalOutput')

    with tile.TileContext(nc) as tc:
        with tc.tile_pool(name='p', bufs=1) as pp:
            x_t = pp.tile([128, NCOLS * F_IN], DT)
            nc.sync.dma_start(x_t[:], x_in.ap())
            deg_t = pp.tile([128, NCOLS], DT)
            nc.scalar.dma_start(deg_t[:], deg_in.ap())

            ideg_t = pp.tile([128, NCOLS], DT)
            nc.vector.reciprocal(ideg_t[:], deg_t[:])
            dis_t = pp.tile([128, NCOLS], DT)
            nc.scalar.sqrt(dis_t[:], ideg_t[:])
            dis16_t = pp.tile([128, NCOLS], BF)
            nc.gpsimd.tensor_copy(dis16_t[:], dis_t[:])

            xs_t = pp.tile([128, NCOLS * F_IN], BF)
            nc.vector.tensor_tensor(
                out=xs_t[:], in0=x_t[:],
                in1=dis_t[:].to_broadcast([128, NCOLS, F_IN]),
                op=mybir.AluOpType.mult)

            nc.sync.dma_start(xs_out.ap(), xs_t[:])
            nc.scalar.dma_start(dis_out.ap(), dis_t[:])
            nc.scalar.dma_start(dis16_out.ap(), dis16_t[:])

    nc.compile()
    return nc


# --------------------------------------------------------------- program d2

def build_d2(cls2, tot2):
    """A1 = seg-sum(xs_src) via strided reduces; As = A1*dis; shuffle to
    [16, NPC]; h1 = relu(W1^T As + b1); zs = dis * (h1^T W2)^T."""
    nc = new_nc()
    xs_in = nc.dram_tensor('xs_pad', [128, tot2], BF, kind='ExternalInput')
    disgp_in = nc.dram_tensor('disgp', [128, NPOS2], BF,
                              kind='ExternalInput')
    disz_in = nc.dram_tensor('disz', [128, 2 * NCOLS], DT,
                             kind='ExternalInput')
    w1_in = nc.dram_tensor('W1', [F_IN, F_HID], DT, kind='ExternalInput')
    w2_in = nc.dram_tensor('W2', [F_HID, F_OUT], DT, kind='ExternalInput')
    b1_in = nc.dram_tensor('b1c', [F_HID, 1], DT, kind='ExternalInput')
    zs_out = nc.dram_tensor('zs', [128, 2 * NCOLS], BF,
                            kind='ExternalOutput')

    offs = class_offsets(cls2)
    # DMA/reduce units: (col0, ncols, pos0, npos, D)
    units = []
    for (P0, n, D), o in zip(cls2, offs):
        nu = max(1, CH2 // D)
        i = 0
        while i < n:
            m = min(nu, n - i)
            units.append((o + i * D, m * D, P0 + i, m, D))
            i += m
    n_waves = 4
    wave_b = [NPOS2 // n_waves * w for w in range(n_waves + 1)]

    with tile.TileContext(nc) as tc:
        with tc.tile_pool(name='p', bufs=1) as pp, \
             tc.tile_pool(name='h1ps', bufs=3, space='PSUM') as h1ps, \
             tc.tile_pool(name='zps', bufs=1, space='PSUM') as zpsp:
            xs_t = pp.tile([128, tot2], BF)
            for i, (c0, nc_, _, _, _) in enumerate(units):
                eng = nc.sync if i % 2 == 0 else nc.scalar
                eng.dma_start(xs_t[:, c0:c0 + nc_],
                              xs_in.ap()[:, c0:c0 + nc_])
            disgp_t = pp.tile([128, NPOS2], BF)
            nc.sync.dma_start(disgp_t[:], disgp_in.ap())
            disz_t = pp.tile([128, 2 * NCOLS], DT)
            nc.scalar.dma_start(disz_t[:], disz_in.ap())
            w1_f32 = pp.tile([F_IN, F_HID], DT)
            nc.sync.dma_start(w1_f32[:], w1_in.ap())
            w1_t = pp.tile([F_IN, F_HID], BF)
            nc.gpsimd.tensor_copy(w1_t[:], w1_f32[:])
            w2_f32 = pp.tile([F_HID, F_OUT], DT)
            nc.scalar.dma_start(w2_f32[:], w2_in.ap())
            w2_t = pp.tile([F_HID, F_OUT], BF)
            nc.gpsimd.tensor_copy(w2_t[:], w2_f32[:])
            b1_t = pp.tile([F_HID, 1], DT)
            nc.sync.dma_start(b1_t[:], b1_in.ap())

            a_raw = pp.tile([128, NPOS2], BF)
            a_s = pp.tile([128, NPOS2], BF)
            with nc.allow_low_precision('fp16 segsum, ~5x error headroom'):
                for c0, nc_, p0, npos, D in units:
                    nc.vector.tensor_reduce(
                        out=a_raw[:, p0:p0 + npos],
                        in_=xs_t[:, c0:c0 + nc_].rearrange(
                            'p (n d) -> p n d', d=D),
                        axis=mybir.AxisListType.X,
                        op=mybir.AluOpType.add)
            for c0, nc_, p0, npos, D in units:
                nc.gpsimd.tensor_tensor(
                    out=a_s[:, p0:p0 + npos], in0=a_raw[:, p0:p0 + npos],
                    in1=disgp_t[:, p0:p0 + npos], op=mybir.AluOpType.mult)

            a_t = pp.tile([F_IN, NPC], BF)
            for w in range(n_waves):
                b0, b1e = wave_b[w], wave_b[w + 1]
                for g in range(8):
                    eng = nc.sync if (w * 8 + g) % 2 == 0 else nc.scalar
                    eng.dma_start(
                        a_t[:, NPOS2 * g + b0:NPOS2 * g + b1e],
                        a_s[F_IN * g:F_IN * (g + 1), b0:b1e])

            h1_sb = pp.tile([F_HID, NPC], BF)
            c = 0
            while c < NPC:
                w = min(512, NPC - c)
                h1p = h1ps.tile([F_HID, 512], DT, space='PSUM', tag='h1')
                nc.tensor.matmul(out=h1p[:, :w], lhsT=w1_t[:],
                                 rhs=a_t[:, c:c + w], start=True, stop=True)
                nc.scalar.activation(h1_sb[:, c:c + w], h1p[:, :w],
                                     mybir.ActivationFunctionType.Relu,
                                     bias=b1_t[:, 0:1])
                c += w

            z_ps = zpsp.tile([128, 2 * NCOLS], DT, space='PSUM')
            for b in range(NCOLS):
                nc.tensor.matmul(out=z_ps[:, 2 * b:2 * b + 2],
                                 lhsT=h1_sb[:, 128 * b:128 * (b + 1)],
                                 rhs=w2_t[:], start=True, stop=True)
            zs_sb = pp.tile([128, 2 * NCOLS], BF)
            nc.vector.tensor_tensor(out=zs_sb[:], in0=z_ps[:],
                                    in1=disz_t[:], op=mybir.AluOpType.mult)
            nc.sync.dma_start(zs_out.ap(), zs_sb[:])

    nc.compile()
    return nc


# --------------------------------------------------------------- program d3

def build_d3(cls3, tot3):
    """A2 = seg-sum(zs_src); out = dis*A2 + b2."""
    nc = new_nc()
    zs_in = nc.dram_tensor('zs_pad', [128, tot3], BF, kind='ExternalInput')
    disr_in = nc.dram_tensor('disr3', [128, 2 * NPOS3], DT,
                             kind='ExternalInput')
    b2_in = nc.dram_tensor('b2rep', [128, 2 * NPOS3], DT,
                           kind='ExternalInput')
    out_out = nc.dram_tensor('out_wrap', [128, 2 * NPOS3], DT,
                             kind='ExternalOutput')

    offs = class_offsets([(P0, n * 2, D) for P0, n, D in cls3])

    with tile.TileContext(nc) as tc:
        with tc.tile_pool(name='p', bufs=1) as pp:
            zs_t = pp.tile([128, tot3], BF)
            for i, ((P0, n, D), o) in enumerate(zip(cls3, offs)):
                eng = nc.sync if i % 2 == 0 else nc.scalar
                eng.dma_start(zs_t[:, o:o + n * 2 * D],
                              zs_in.ap()[:, o:o + n * 2 * D])
            disr_t = pp.tile([128, 2 * NPOS3], DT)
            nc.sync.dma_start(disr_t[:], disr_in.ap())
            b2_t = pp.tile([128, 2 * NPOS3], DT)
            nc.scalar.dma_start(b2_t[:], b2_in.ap())

            agg = pp.tile([128, 2 * NPOS3], BF)
            with nc.allow_low_precision('fp16 segsum, ~5x error headroom'):
                for (P0, n, D), o in zip(cls3, offs):
                    nc.vector.tensor_reduce(
                        out=agg[:, 2 * P0:2 * (P0 + n)],
                        in_=zs_t[:, o:o + n * 2 * D].rearrange(
                            'p (n d) -> p n d', d=D),
                        axis=mybir.AxisListType.X,
                        op=mybir.AluOpType.add)
            t1 = pp.tile([128, 2 * NPOS3], DT)
            nc.vector.tensor_tensor(out=t1[:], in0=agg[:], in1=disr_t[:],
                                    op=mybir.AluOpType.mult)
            out_t = pp.tile([128, 2 * NPOS3], DT)
            nc.gpsimd.tensor_tensor(out=out_t[:], in0=t1[:], in1=b2_t[:],
                                    op=mybir.AluOpType.add)
            nc.sync.dma_start(out_out.ap(), out_t[:])

    nc.compile()
    return nc


# ------------------------------------------------------------------ runner

RESULTS = []


def run_gcn(x, edge_index, W1, b1, W2, b2, trace=False):
    x = np.asarray(x, dtype=np.float32)
    W1 = np.asarray(W1, dtype=np.float32)
    b1 = np.asarray(b1, dtype=np.float32)
    W2 = np.asarray(W2, dtype=np.float32)
    b2 = np.asarray(b2, dtype=np.float32)

    sch = build_schedule(edge_index)
    cls2, cls3 = sch['cls2'], sch['cls3']
    tot2, tot3 = sch['tot2'], sch['tot3']
    print(f'[host] tot2={tot2} tot3={tot3} cls2={cls2} cls3={cls3}')

    import time
    t0 = time.time()
    nc0 = build_d0()
    nc2 = build_d2(cls2, tot2)
    nc3 = build_d3(cls3, tot3)
    print(f'[host] compiled in {time.time()-t0:.1f}s')

    idx2 = [build_idx2(sch, c) for c in range(N_CORES)]
    idx3 = [build_idx3(sch, c) for c in range(N_CORES)]

    core_ids = list(range(N_CORES))
    times = {}
    RESULTS.clear()

    # ---------- d0
    x_pad = np.zeros((NPAD, F_IN), dtype=np.float32)
    x_pad[:N_NODES] = x
    deg_f = sch['deg'].astype(np.float32)
    deg_f[N_NODES:] = 1.0
    in0 = []
    for c in range(N_CORES):
        lo = c * NPC
        xw = np.ascontiguousarray(
            x_pad[lo:lo + NPC].reshape(NCOLS, 128, F_IN).transpose(1, 0, 2)
            .reshape(128, NCOLS * F_IN))
        in0.append({'x_wrap': xw, 'deg_wrap': wrap2(deg_f[lo:lo + NPC])})
    r0 = run_bass_kernel_spmd(nc0, in0, core_ids=core_ids, trace=trace)
    RESULTS.append(r0)
    times['d0'] = r0.exec_time_ns

    xs_full = np.zeros((NPAD + 1, F_IN), dtype=NP_BF)
    dis_full = np.empty(NPAD, dtype=np.float32)
    dis16_full = np.empty(NPAD, dtype=NP_BF)
    for c in range(N_CORES):
        lo = c * NPC
        xs_full[lo:lo + NPC] = (r0.results[c]['xs_bf']
                                .reshape(128, NCOLS, F_IN).transpose(1, 0, 2)
                                .reshape(NPC, F_IN))
        dis_full[lo:lo + NPC] = unwrap2(r0.results[c]['dis'])
        dis16_full[lo:lo + NPC] = unwrap2(r0.results[c]['dis16'])
    xs_full[N_NODES:] = 0

    # ---------- d2 host inputs
    b1c = np.ascontiguousarray(b1[:, None])
    in2 = []
    for c in range(N_CORES):
        lo = c * NPC
        rank_nodes = sch['cores'][c]['rank_nodes']
        xs_pad = gather2(xs_full, idx2[c], tot2)
        # disgp[16g+f, pos] = dis16[node(g,pos)];  node(g,pos)=rank[8*pos+g]
        nodemat = rank_nodes.reshape(NPOS2, 8).T          # [8, NPOS2]
        disgp = np.repeat(dis16_full[lo + nodemat], F_IN, axis=0)
        # disz[i, 2b+q] = dis[node(ct=128b+i)], ct = NPOS2*g + pos
        ct = (128 * np.arange(NCOLS)[None, :]
              + np.arange(128)[:, None])                  # [128, 98]
        g, pos = ct // NPOS2, ct % NPOS2
        node_ct = rank_nodes[8 * pos + g]                 # [128, 98]
        disz = np.repeat(dis_full[lo + node_ct], F_OUT,
                         axis=1).reshape(128, 2 * NCOLS)
        in2.append({'xs_pad': xs_pad,
                    'disgp': np.ascontiguousarray(disgp),
                    'disz': np.ascontiguousarray(disz),
                    'W1': W1, 'W2': W2, 'b1c': b1c})
    r2 = run_bass_kernel_spmd(nc2, in2, core_ids=core_ids, trace=trace)
    RESULTS.append(r2)
    times['d2'] = r2.exec_time_ns

    zs_full = np.zeros((NPAD + 1, F_OUT), dtype=NP_BF)
    for c in range(N_CORES):
        lo = c * NPC
        rank_nodes = sch['cores'][c]['rank_nodes']
        ct = (128 * np.arange(NCOLS)[None, :] + np.arange(128)[:, None])
        g, pos = ct // NPOS2, ct % NPOS2
        node_ct = rank_nodes[8 * pos + g]
        zs = r2.results[c]['zs'].reshape(128, NCOLS, F_OUT)
        zs_full[lo + node_ct.reshape(-1)] = zs.reshape(-1, F_OUT)
    zs_full[N_NODES:] = 0

    # ---------- d3 host inputs
    b2rep = np.ascontiguousarray(
        np.broadcast_to(b2[None, None, :], (128, NPOS3, F_OUT))
        .reshape(128, 2 * NPOS3)).astype(np.float32)
    in3 = []
    for c in range(N_CORES):
        lo = c * NPC
        rank_nodes = sch['cores'][c]['rank_nodes']
        zs_pad = gather3(zs_full, idx3[c], tot3)
        nodemat3 = rank_nodes.reshape(NPOS3, 128).T       # [128, NPOS3]
        disr3 = np.repeat(dis_full[lo + nodemat3], F_OUT,
                          axis=1).reshape(128, 2 * NPOS3)
        in3.append({'zs_pad': zs_pad,
                    'disr3': np.ascontiguousarray(disr3),
                    'b2rep': b2rep})
    r3 = run_bass_kernel_spmd(nc3, in3, core_ids=core_ids, trace=trace)
    RESULTS.append(r3)
    times['d3'] = r3.exec_time_ns

    out_full = np.empty((NPAD, F_OUT), dtype=np.float32)
    for c in range(N_CORES):
        lo = c * NPC
        rank_nodes = sch['cores'][c]['rank_nodes']
        ow = r3.results[c]['out_wrap'].reshape(128, NPOS3, F_OUT)
        nodemat3 = rank_nodes.reshape(NPOS3, 128).T
        out_full[lo + nodemat3.reshape(-1)] = ow.reshape(-1, F_OUT)
    return out_full[:N_NODES].astype(np.float32), times


# ------------------------------------------------------------- entry point

TRACE = False
LAST_TIMES = {}


def kernel(x, edge_index, W1, b1, W2, b2):
    """Full-input GCN kernel: shards across 8 NeuronCores internally."""
    global LAST_TIMES
    out, times = run_gcn(x, edge_index, W1, b1, W2, b2, trace=TRACE)
    LAST_TIMES = times
    return out
